# revision 1
# baseline (speedup 1.0000x reference)
"""Trainium2 Bass kernel for the quirky MultiHeadAttention module.

Reference computation (B=4, S=1024, H=768, NH=12, HS=64):
    Q = (x@Wq+bq)  split into heads     [B,12,S,64]
    K = (x@Wk+bk)  split into heads     [B,12,S,64]
    V = x@Wv+bv    NOT split            [B,S,768]
    A = softmax(QK^T/8 + mask)          [B,12,S,S]
    out = (A @ V) reshaped [B, S*12, H] @ Wo + bo    -> [4, 12288, 768]

Algebraic restructuring used here:
  * (A @ V) @ Wo = A @ (V @ Wo) = A @ (x @ (Wv@Wo) + 1x(bv@Wo)); with bo
    folded in, each output row is A[q,:] @ VW + c, c = bv@Wo + bo, and the
    +c term is realized exactly by adding a constant row to VW (softmax
    rows sum to one in exact correspondence with the sigma column below).
  * Masked keys produce exp(-1e9+s) == 0 in fp32 for every head and every
    query (the mask is [B,1,1,S]), identically in the reference, so masked
    keys are dropped entirely on the host and the key axis is compacted
    (~2x less attention work for a Bernoulli(1/2) mask).
  * The softmax denominator comes from a ones-column appended to VW, and
    exp needs no max-subtraction (scores are O(1) for this problem).

Sharding: 8 cores = 4 batches x 2 head-groups (6 heads each). Pure SPMD,
no collectives. Everything is computed in a transposed layout so no
on-device transposes are needed:
    QT/KT: [384 feat, tok] (head-pairs packed 64+64 in partitions; the
        64-row score matmuls are row-packed on the PE via tile_position)
    S^T = KT_h-slices.T @ QT_h  -> [k, q]  (k on partitions => the mask is
        a per-partition bias folded into the Exp activation for free)
    U = exp(S^T)  [k, q] fp16   -> exactly the layout the PV matmul needs
    O = U.T @ [VW | 1]  -> [q, 769] with col 768 = softmax denominator
Matmul operands are fp16 (same PE speed as bf16 on TRN2, ~4x less rounding
error); all accumulation is fp32 in PSUM. Inputs are packed host-side into
partition-major [128, N] blobs so the input DMAs run long contiguous lines,
and a burst of dummy matmuls during the initial DMA wait pre-warms the PE
clock (HAM) to 2.4 GHz.
"""

import math

import numpy as np

B, S, H, NH, HS = 4, 1024, 768, 12, 64
GW = 384          # head-group width = 6 heads * 64
NCORES = 8

_PROGRAM_CACHE = {}


def _pack6(a):
    """[768, N] -> partition-major [128, 6*N] (tile i at cols i*N:(i+1)*N)."""
    n = a.shape[1]
    return np.ascontiguousarray(
        a.reshape(6, 128, n).transpose(1, 0, 2).reshape(128, 6 * n))


def _build_program(kt_tiles, has_cvec):
    """kt_tiles: number of 128-wide compacted-key tiles (1..8).
    has_cvec: include the rank-1 (bv@Wo + bo) constant row in VW."""
    import concourse.mybir as mybir
    import concourse.tile as tile
    from concourse import bacc
    from concourse.bass import ds, ts

    f32 = mybir.dt.float32
    f16 = mybir.dt.float16
    AF = mybir.ActivationFunctionType

    KMAX = 128 * kt_tiles
    # key chunks (<=512 wide, balanced) for the KT projection
    if KMAX <= 512:
        kchunks = [(0, KMAX)]
    else:
        w1 = 128 * ((kt_tiles + 1) // 2)
        kchunks = [(0, w1), (w1, KMAX - w1)]

    nc = bacc.Bacc(None, target_bir_lowering=False, debug=False)

    xp_d = nc.dram_tensor("xp", (128, 6 * 1024), f16, kind="ExternalInput")
    wqp_d = nc.dram_tensor("wqp", (128, 6 * 384), f16, kind="ExternalInput")
    wkp_d = nc.dram_tensor("wkp", (128, 6 * 384), f16, kind="ExternalInput")
    wvp_d = nc.dram_tensor("wvp", (128, 6 * 768), f16, kind="ExternalInput")
    wvo6_d = nc.dram_tensor("wvo6", (1, 768), f16, kind="ExternalInput")
    # small fp32 per-partition vectors: cols = bq(3) bk(3) mk(kt_tiles)
    sv_d = nc.dram_tensor("sv", (128, 6 + kt_tiles), f32, kind="ExternalInput")
    out_d = nc.dram_tensor("out", (6, 1024, 768), f32, kind="ExternalOutput")

    with tile.TileContext(nc) as tc:
        with (
            tc.tile_pool(name="persist", bufs=1) as pp,
            tc.tile_pool(name="ut", bufs=4 * kt_tiles) as utp,
            tc.tile_pool(name="eps", bufs=8) as ep,
            tc.tile_pool(name="osb", bufs=4) as op_,
        ):
            # ---- stream inputs into SBUF (order = load priority) ----
            sv = pp.tile([128, 6 + kt_tiles], f32, name="sv", tag="sv")
            nc.sync.dma_start(sv[:], sv_d[:])
            bq_t = [sv[:, j:j + 1] for j in range(3)]
            bk_t = [sv[:, 3 + j:4 + j] for j in range(3)]
            mk_t = [sv[:, 6 + k:7 + k] for k in range(kt_tiles)]

            xbig = pp.tile([128, 6 * 1024], f16, name="xbig", tag="xbig")
            wqbig = pp.tile([128, 6 * 384], f16, name="wqbig", tag="wqbig")
            wkbig = pp.tile([128, 6 * 384], f16, name="wkbig", tag="wkbig")
            wvbig = pp.tile([128, 6 * 768], f16, name="wvbig", tag="wvbig")
            xkt6 = pp.tile([1, KMAX], f16, name="xkt6", tag="xkt6")
            wvo6 = pp.tile([1, 768], f16, name="wvo6", tag="wvo6")
            # Input loads: fine-grained pieces alternating over the two
            # HWDGE rings (sync, scalar) in consumption order, VW weights
            # on SWDGE (gpsimd). Small pieces land early so the first QT
            # matmuls can start while the rest of x streams in.
            rings = [nc.sync, nc.scalar]
            wh = 3 * 384
            for r in range(2):
                rings[r].dma_start(wqbig[:, r * wh:(r + 1) * wh],
                                   wqp_d[:, r * wh:(r + 1) * wh])
            for i in range(6):
                rings[i % 2].dma_start(xbig[:, i * 1024:(i + 1) * 1024],
                                       xp_d[:, i * 1024:(i + 1) * 1024])
            for r in range(2):
                rings[r].dma_start(wkbig[:, r * wh:(r + 1) * wh],
                                   wkp_d[:, r * wh:(r + 1) * wh])
            if has_cvec:
                nc.vector.memset(xkt6[:], 1.0)
                nc.scalar.dma_start(wvo6[:], wvo6_d[:])
            nc.gpsimd.dma_start(wvbig[:], wvp_d[:])

            xt = [xbig[:, i * 1024:(i + 1) * 1024] for i in range(6)]
            wq_t = [wqbig[:, i * 384:(i + 1) * 384] for i in range(6)]
            # tokens are host-permuted (kept keys first), so the K-side
            # tiles are just the leading columns of the same x buffer
            xkt = [xbig[:, i * 1024:i * 1024 + KMAX] for i in range(6)]
            wk_t = [wkbig[:, i * 384:(i + 1) * 384] for i in range(6)]
            wvo_t = [wvbig[:, i * 768:(i + 1) * 768] for i in range(6)]

            # persistent intermediates
            QT = [pp.tile([128, 1024], f16, name=f"QT{j}", tag=f"QT{j}")
                  for j in range(3)]
            KT = [pp.tile([128, KMAX], f16, name=f"KT{j}", tag=f"KT{j}")
                  for j in range(3)]
            VW = [pp.tile([128, 769], f16, name=f"VW{m}", tag=f"VW{m}")
                  for m in range(kt_tiles)]

            # ---- phase A: projections ----
            # PE warm-up: dummy matmuls on a tiny memset tile keep the
            # tensor engine active during the initial input DMA so the
            # HAM clock gate opens (2.4 GHz) before real work arrives.
            wsrc = pp.tile([1, 512], f16, name="wsrc", tag="wsrc")
            nc.vector.memset(wsrc[:], 0.0)
            with tc.tile_pool(name="psW", bufs=2, space="PSUM") as psW:
                for _ in range(12):
                    psw = psW.tile([1, 512], f32, name="warm", tag="warm")
                    nc.tensor.matmul(psw[:], wsrc[:, 0:1], wsrc[:])

            with tc.tile_pool(name="psA", bufs=6, space="PSUM") as psA:
                # QT is kt-major: all six (j,qc) PSUM groups accumulate in
                # parallel so each arriving x tile is consumed immediately
                # (no long PE stalls while x streams in).
                qgroups = [(j, qc) for j in range(3) for qc in range(2)]
                qps = [psA.tile([128, 512], f32, name=f"qtp{j}{qc}", tag="qk")
                       for j, qc in qgroups]
                for kt in range(6):
                    for gi, (j, qc) in enumerate(qgroups):
                        nc.tensor.matmul(
                            qps[gi][:], wq_t[kt][:, ts(j, 128)],
                            xt[kt][:, ds(qc * 512, 512)],
                            start=(kt == 0), stop=(kt == 5))
                for gi, (j, qc) in enumerate(qgroups):
                    nc.scalar.activation(
                        QT[j][:, ds(qc * 512, 512)], qps[gi][:], AF.Identity,
                        bias=bq_t[j])
                for j in range(3):
                    for o, w in kchunks:
                        kch = ds(o, w)
                        ps2 = psA.tile([128, 512], f32, name="ktp", tag="qk")
                        for kt in range(6):
                            nc.tensor.matmul(
                                ps2[:, 0:w], wk_t[kt][:, ts(j, 128)],
                                xkt[kt][:, kch],
                                start=(kt == 0), stop=(kt == 5))
                        nc.scalar.activation(
                            KT[j][:, kch], ps2[:, 0:w], AF.Identity,
                            bias=bk_t[j])
                for m in range(kt_tiles):   # compacted-key token tile
                    for ncn in range(2):    # output feature chunk of 384
                        fch = ds(ncn * 384, 384)
                        ps = psA.tile([128, 384], f32, name="vw", tag="vw",
                                      bufs=2)
                        for kt in range(6):
                            nc.tensor.matmul(
                                ps[:], xkt[kt][:, ts(m, 128)], wvo_t[kt][:, fch],
                                start=(kt == 0),
                                stop=(kt == 5 and not has_cvec))
                        if has_cvec:
                            nc.tensor.matmul(
                                ps[:], xkt6[:, ts(m, 128)], wvo6[:, fch],
                                start=False, stop=True)
                        nc.vector.tensor_copy(VW[m][:, fch], ps[:])
                    nc.vector.memset(VW[m][:, 768:769], 1.0)

            # ---- phase B: attention ----
            with (
                tc.tile_pool(name="psS", bufs=4, space="PSUM") as psSp,
                tc.tile_pool(name="psO", bufs=2, space="PSUM") as psOp,
            ):
                chunks = [(j, qc) for j in range(3) for qc in range(2)]

                def emit_scores(j, qc):
                    qch = ds(qc * 512, 512)
                    ut = [[None] * kt_tiles for _ in range(2)]
                    for kt in range(kt_tiles):
                        for hh in range(2):
                            p0 = hh * 64
                            ps = psSp.tile([128, 512], f32, name="psS",
                                           tag="psS")
                            # 64-row-packed scores^T: [k-tile, q-chunk]
                            nc.tensor.matmul(
                                ps[:],
                                KT[j][p0:p0 + 64, ts(kt, 128)],
                                QT[j][p0:p0 + 64, qch])
                            u = utp.tile([128, 512], f16, name="ut", tag="ut")
                            nc.scalar.activation(
                                u[:], ps[:], AF.Exp, bias=mk_t[kt])
                            ut[hh][kt] = u
                    return ut

                for ci, (j, qc) in enumerate(chunks):
                    ut = emit_scores(j, qc)
                    for gi, (hh, mq) in enumerate(
                            (hh, mq) for hh in range(2) for mq in range(4)):
                        head = j * 2 + hh
                        # 384+385 split: both PV chains stream ~160ns/MM so
                        # LDWEIGHTS (~97ns) stays fully hidden; sigma-chain
                        # first so the reciprocal overlaps the other chain.
                        pa = psOp.tile([128, 384], f32, name="psOa",
                                       tag="psOa")
                        pb = psOp.tile([128, 385], f32, name="psOb",
                                       tag="psOb")
                        for kt in range(kt_tiles):
                            nc.tensor.matmul(
                                pb[:], ut[hh][kt][:, ts(mq, 128)],
                                VW[kt][:, 384:769],
                                start=(kt == 0), stop=(kt == kt_tiles - 1))
                        for kt in range(kt_tiles):
                            nc.tensor.matmul(
                                pa[:], ut[hh][kt][:, ts(mq, 128)],
                                VW[kt][:, 0:384],
                                start=(kt == 0), stop=(kt == kt_tiles - 1))
                        rv = ep.tile([128, 1], f32, name="rinv", tag="rinv")
                        nc.vector.reciprocal(rv[:], pb[:, 384:385])
                        ob = op_.tile([128, 768], f32, name="ob", tag="ob")
                        orow = out_d[head, ds(qc * 512 + mq * 128, 128), :]
                        nc.vector.tensor_scalar_mul(
                            ob[:, 384:768], pb[:, 0:384], rv[:])
                        nc.scalar.dma_start(orow[:, 384:768], ob[:, 384:768])
                        nc.vector.tensor_scalar_mul(
                            ob[:, 0:384], pa[:], rv[:])
                        nc.sync.dma_start(orow[:, 0:384], ob[:, 0:384])
    nc.compile()
    return nc


def get_program(kt_tiles=8, has_cvec=True):
    key = (kt_tiles, has_cvec)
    if key not in _PROGRAM_CACHE:
        _PROGRAM_CACHE[key] = _build_program(*key)
    return _PROGRAM_CACHE[key]


def prep(x, mask, Wq, bq, Wk, bk, Wv, bv, Wo, bo):
    """Host-side sharding/compaction.
    Tokens are permuted per batch so unmasked keys come first; the device
    computes everything in permuted token order and gather_output undoes
    the permutation. Returns (kt_tiles, has_cvec, in_maps, perms)."""
    f16 = np.float16
    x = np.asarray(x, np.float32)
    mask = np.asarray(mask)
    Wq = np.asarray(Wq, np.float32)
    Wk = np.asarray(Wk, np.float32)
    Wv = np.asarray(Wv, np.float32)
    Wo = np.asarray(Wo, np.float32)
    bq = np.asarray(bq, np.float32)
    bk = np.asarray(bk, np.float32)
    bv = np.asarray(bv, np.float32)
    bo = np.asarray(bo, np.float32)

    mrow = [mask[b, 0, 0] != 0 for b in range(B)]
    perms = [np.argsort(~mrow[b], kind="stable") for b in range(B)]
    nkeep = [int(mrow[b].sum()) for b in range(B)]
    kt_tiles = min(8, max(1, math.ceil(max(nkeep) / 128)))
    KMAX = 128 * kt_tiles

    cvec = bv @ Wo + bo
    has_cvec = bool(np.any(cvec))

    # per-head-group packed weights (shared across the 4 batches)
    wq_p, wk_p, bq_p, bk_p = [], [], [], []
    for g in range(2):
        cs = slice(g * GW, (g + 1) * GW)
        wq_p.append(_pack6((Wq[:, cs] * 0.125).astype(f16)))
        wk_p.append(_pack6(Wk[:, cs].astype(f16)))
        bq_p.append((bq[cs] * 0.125).reshape(3, 128).T)   # [128,3]
        bk_p.append(bk[cs].reshape(3, 128).T)
    wvp = _pack6((Wv @ Wo).astype(f16))
    wvo6 = cvec.astype(f16).reshape(1, 768)

    xp_b, sv_b = [], []
    for b in range(B):
        xp_b.append(_pack6(x[b][perms[b]].T.astype(f16)))
        sv = np.empty((128, 6 + kt_tiles), np.float32)
        mk = np.full(KMAX, -1e9, np.float32)
        mk[:nkeep[b]] = 0.0
        sv[:, 6:] = mk.reshape(kt_tiles, 128).T
        sv_b.append(sv)

    in_maps = []
    for c in range(NCORES):
        b, g = c // 2, c % 2
        sv = sv_b[b].copy()
        sv[:, 0:3] = bq_p[g]
        sv[:, 3:6] = bk_p[g]
        in_maps.append({
            "xp": xp_b[b],
            "wqp": wq_p[g],
            "wkp": wk_p[g],
            "wvp": wvp,
            "wvo6": wvo6,
            "sv": sv,
        })
    return kt_tiles, has_cvec, in_maps, perms


def gather_output(results, perms):
    out = np.empty((B, S * NH, H), np.float32)
    ov = out.reshape(B, S, NH, H)
    for c in range(NCORES):
        b, g = c // 2, c % 2
        o = results[c]["out"]  # [6, 1024(permuted q), 768]
        ov[b, perms[b], g * 6:(g + 1) * 6, :] = o.transpose(1, 0, 2)
    return out


def kernel(**inputs):
    from concourse.bass_utils import run_bass_kernel_spmd

    kt_tiles, has_cvec, in_maps, perms = prep(**inputs)
    nc = get_program(kt_tiles, has_cvec)
    res = run_bass_kernel_spmd(nc, in_maps, core_ids=list(range(NCORES)))
    return gather_output(res.results, perms)


if __name__ == "__main__":
    rng = np.random.default_rng(0)
    demo = {
        "x": rng.standard_normal((B, S, H), dtype=np.float32),
        "mask": rng.integers(0, 2, (B, 1, 1, S)).astype(np.int32),
        "Wq": rng.standard_normal((H, H), dtype=np.float32) / np.sqrt(H),
        "bq": np.zeros(H, np.float32),
        "Wk": rng.standard_normal((H, H), dtype=np.float32) / np.sqrt(H),
        "bk": np.zeros(H, np.float32),
        "Wv": rng.standard_normal((H, H), dtype=np.float32) / np.sqrt(H),
        "bv": np.zeros(H, np.float32),
        "Wo": rng.standard_normal((H, H), dtype=np.float32) / np.sqrt(H),
        "bo": np.zeros(H, np.float32),
    }
    out = kernel(**demo)
    print("kernel ran, output shape", out.shape)



# revision 6
# speedup vs baseline: 1.0074x; 1.0074x over previous
"""Trainium2 Bass kernel for the quirky MultiHeadAttention module.

Reference computation (B=4, S=1024, H=768, NH=12, HS=64):
    Q = (x@Wq+bq)  split into heads     [B,12,S,64]
    K = (x@Wk+bk)  split into heads     [B,12,S,64]
    V = x@Wv+bv    NOT split            [B,S,768]
    A = softmax(QK^T/8 + mask)          [B,12,S,S]
    out = (A @ V) reshaped [B, S*12, H] @ Wo + bo    -> [4, 12288, 768]

Algebraic restructuring:
  * (A @ V) @ Wo = A @ (V @ Wo); with zero-able bias fold (bv@Wo+bo as a
    rank-1 constant row, exact because softmax rows sum to 1 against the
    sigma column below).
  * Masked keys are dropped host-side and the key axis compacted (~2x
    less attention work for a Bernoulli(1/2) mask).
  * Softmax denominator = a ones-column appended to VW; no max-subtract.

Layouts (per core: one batch x 6-head group, pure SPMD, no collectives):
    QT/KT: [384 feat, tok]; S^T = KT_h.T @ QT_h -> [k, q] so the mask is a
    per-partition Exp bias; U = exp(S^T) fp16; O = U.T @ [VW | 1] -> [q,769].

Perf structure (vs the 149.6us baseline):
  * Output written fp16 (151->75MB total) and upcast on host: removes the
    output-DMA backpressure that stalled the PE near the end.
  * Overflow-key row packing: keys beyond 4 full 128-tiles (<=32 of them
    for this data) go in a 32-slot group replicated 4x across partitions;
    the per-head overflow PV matmuls address disjoint PE row groups
    (tile_position) so adjacent heads' overflow MMs run concurrently.
    PV cost ~ 4.1 key tiles instead of 5.
  * Cross-chunk software pipelining: chunk c+1's score MMs + exps are
    sprinkled between chunk c's PV groups, so PV never waits on exp.
  * HAM clock management: dense dummy matmuls from t=0 until real work
    arrives keep the PE activity window busy so the 2.4 GHz clock gate
    opens ~4us in and never re-throttles (idle >3.4us closes it).
  * Input DMA: x in 12 fine pieces on the two HWDGE rings (wq halves
    first), wk+wvp on the gpsimd SWDGE ring; output DMA alternates
    sync/gpsimd rings, one contiguous [128,768] fp16 transfer per combo.
"""

import math

import numpy as np

B, S, H, NH, HS = 4, 1024, 768, 12, 64
GW = 384          # head-group width = 6 heads * 64
NCORES = 8

_PROGRAM_CACHE = {}


def _pack6(a):
    """[768, N] -> partition-major [128, 6*N] (tile i at cols i*N:(i+1)*N)."""
    n = a.shape[1]
    return np.ascontiguousarray(
        a.reshape(6, 128, n).transpose(1, 0, 2).reshape(128, 6 * n))


def _build_program(kt_full, ov, has_cvec):
    """kt_full: number of full 128-wide compacted-key tiles.
    ov: overflow group width (0 = none, 32 or 64); overflow keys live in a
    [128, .] stack replicated 128//ov times so per-head overflow PV MMs can
    target disjoint PE row groups and run concurrently.
    has_cvec: include the rank-1 (bv@Wo + bo) constant row in VW."""
    import concourse.mybir as mybir
    import concourse.tile as tile
    from concourse import bacc
    from concourse.bass import ds, ts

    f32 = mybir.dt.float32
    f16 = mybir.dt.float16
    AF = mybir.ActivationFunctionType

    KF = 128 * kt_full            # full-tile key span
    nt = kt_full + (1 if ov else 0)   # tiles incl. overflow stack
    # key chunks (<=512 wide) of the full span for the KT projection
    kchunks = []
    o = 0
    while o < KF:
        w = min(512, KF - o)
        kchunks.append((o, w))
        o += w

    nc = bacc.Bacc(None, target_bir_lowering=False, debug=False)

    xp_d = nc.dram_tensor("xp", (128, 6 * 1024), f16, kind="ExternalInput")
    wqp_d = nc.dram_tensor("wqp", (128, 6 * 384), f16, kind="ExternalInput")
    wkp_d = nc.dram_tensor("wkp", (128, 6 * 384), f16, kind="ExternalInput")
    wvp_d = nc.dram_tensor("wvp", (128, 6 * 768), f16, kind="ExternalInput")
    wvo6_d = nc.dram_tensor("wvo6", (1, 768), f16, kind="ExternalInput")
    xk5_d = (nc.dram_tensor("xk5", (128, 6 * 128), f16, kind="ExternalInput")
             if ov else None)
    # small fp32 per-partition vectors: cols = bq(3) bk(3) mk(kt_full) mk5(0/1)
    sv_d = nc.dram_tensor("sv", (128, 6 + nt), f32, kind="ExternalInput")
    out_d = nc.dram_tensor("out", (6, 1024, 768), f16, kind="ExternalOutput")

    with tile.TileContext(nc) as tc:
        with (
            tc.tile_pool(name="persist", bufs=1) as pp,
            tc.tile_pool(name="ut", bufs=2 * (2 * kt_full + (1 if ov else 0))) as utp,
            tc.tile_pool(name="eps", bufs=8) as ep,
            tc.tile_pool(name="osb", bufs=4) as op_,
        ):
            # ---- stream inputs into SBUF (order = load priority) ----
            sv = pp.tile([128, 6 + nt], f32, name="sv", tag="sv")
            nc.sync.dma_start(sv[:], sv_d[:])
            bq_t = [sv[:, j:j + 1] for j in range(3)]
            bk_t = [sv[:, 3 + j:4 + j] for j in range(3)]
            mk_t = [sv[:, 6 + k:7 + k] for k in range(kt_full)]
            mk5 = sv[:, 6 + kt_full:7 + kt_full] if ov else None

            xbig = pp.tile([128, 6 * 1024], f16, name="xbig", tag="xbig")
            wqbig = pp.tile([128, 6 * 384], f16, name="wqbig", tag="wqbig")
            wkbig = pp.tile([128, 6 * 384], f16, name="wkbig", tag="wkbig")
            wvbig = pp.tile([128, 6 * 768], f16, name="wvbig", tag="wvbig")
            xk5 = (pp.tile([128, 6 * 128], f16, name="xk5", tag="xk5")
                   if ov else None)
            xkt6 = pp.tile([1, 128], f16, name="xkt6", tag="xkt6")
            wvo6 = pp.tile([1, 768], f16, name="wvo6", tag="wvo6")
            # Input loads: wq halves first on the two HWDGE rings, then x in
            # twelve 512-col pieces (ring = query-half so both halves of a
            # kt tile land together), wk+wvp on the SWDGE ring.
            rings = [nc.sync, nc.scalar]
            wh = 3 * 384
            for r in range(2):
                rings[r].dma_start(wqbig[:, r * wh:(r + 1) * wh],
                                   wqp_d[:, r * wh:(r + 1) * wh])
            for kt in range(6):
                for qc in range(2):
                    cs = ds(kt * 1024 + qc * 512, 512)
                    rings[qc].dma_start(xbig[:, cs], xp_d[:, cs])
            nc.gpsimd.dma_start(wkbig[:], wkp_d[:])
            if ov:
                nc.scalar.dma_start(xk5[:], xk5_d[:])
            if has_cvec:
                nc.vector.memset(xkt6[:], 1.0)
                nc.sync.dma_start(wvo6[:], wvo6_d[:])
            nc.gpsimd.dma_start(wvbig[:], wvp_d[:])

            xt = [xbig[:, i * 1024:(i + 1) * 1024] for i in range(6)]
            wq_t = [wqbig[:, i * 384:(i + 1) * 384] for i in range(6)]
            # tokens are host-permuted (kept keys first), so the K-side
            # tiles are the leading columns of the same x buffer
            xkt = [xbig[:, i * 1024:i * 1024 + KF] for i in range(6)]
            xk5t = ([xk5[:, i * 128:(i + 1) * 128] for i in range(6)]
                    if ov else None)
            wk_t = [wkbig[:, i * 384:(i + 1) * 384] for i in range(6)]
            wvo_t = [wvbig[:, i * 768:(i + 1) * 768] for i in range(6)]

            # persistent intermediates
            KW = KF + (128 if ov else 0)
            QT = [pp.tile([128, 1024], f16, name=f"QT{j}", tag=f"QT{j}")
                  for j in range(3)]
            KT = [pp.tile([128, KW], f16, name=f"KT{j}", tag=f"KT{j}")
                  for j in range(3)]
            VW = [pp.tile([128, 769], f16, name=f"VW{m}", tag=f"VW{m}")
                  for m in range(nt)]

            # ---- PE warm-up ----
            # Dense dummy matmuls keep the PE activity window busy from t=0
            # so the HAM clock gate opens (~3.4us of sustained activity) and
            # real work runs at 2.4 GHz from the start.
            wsrc = pp.tile([128, 512], f16, name="wsrc", tag="wsrc")
            nc.vector.memset(wsrc[:], 0.0)
            with tc.tile_pool(name="psW", bufs=2, space="PSUM") as psW:
                for _ in range(14):
                    psw = psW.tile([1, 512], f32, name="warm", tag="warm")
                    nc.tensor.matmul(psw[:], wsrc[:, 0:1], wsrc[:])

            # ---- phase A: projections ----
            with tc.tile_pool(name="psA", bufs=6, space="PSUM") as psA:
                # QT kt-major: all six (j,qc) PSUM groups accumulate in
                # parallel so each arriving x piece is consumed immediately.
                qgroups = [(j, qc) for qc in range(2) for j in range(3)]
                qps = [psA.tile([128, 512], f32, name=f"qtp{j}{qc}", tag="qk")
                       for j, qc in qgroups]
                for kt in range(6):
                    for gi, (j, qc) in enumerate(qgroups):
                        nc.tensor.matmul(
                            qps[gi][:], wq_t[kt][:, ts(j, 128)],
                            xt[kt][:, ds(qc * 512, 512)],
                            start=(kt == 0), stop=(kt == 5))
                for gi, (j, qc) in enumerate(qgroups):
                    nc.scalar.activation(
                        QT[j][:, ds(qc * 512, 512)], qps[gi][:], AF.Identity,
                        bias=bq_t[j])
                for j in range(3):
                    for o, w in kchunks:
                        kch = ds(o, w)
                        ps2 = psA.tile([128, 512], f32, name="ktp", tag="qk")
                        for kt in range(6):
                            nc.tensor.matmul(
                                ps2[:, 0:w], wk_t[kt][:, ts(j, 128)],
                                xkt[kt][:, kch],
                                start=(kt == 0), stop=(kt == 5))
                        nc.scalar.activation(
                            KT[j][:, kch], ps2[:, 0:w], AF.Identity,
                            bias=bk_t[j])
                    if ov:
                        ps2 = psA.tile([128, 128], f32, name="ktp5",
                                       tag="qk5", bufs=2)
                        for kt in range(6):
                            nc.tensor.matmul(
                                ps2[:], wk_t[kt][:, ts(j, 128)], xk5t[kt][:],
                                start=(kt == 0), stop=(kt == 5))
                        nc.scalar.activation(
                            KT[j][:, ds(KF, 128)], ps2[:], AF.Identity,
                            bias=bk_t[j])

            # scores for the first chunk are emitted before VW so their exps
            # overlap the VW matmuls (cross-chunk pipeline warm-up); VW then
            # runs, then the PV loop with next-chunk scores sprinkled in.
            chunks = [(j, qc) for j in range(3) for qc in range(2)]
            nrep = (128 // ov) if ov else 0

            with tc.tile_pool(name="psS", bufs=4, space="PSUM") as psSp:
                def emit_score_unit(ci, kt):
                    """One (kt) pair: both heads' score MM + exp. The two MMs
                    address PE rows 0-63 / 64-127 so they run concurrently."""
                    j, qc = chunks[ci]
                    qch = ds(qc * 512, 512)
                    out = []
                    for hh in range(2):
                        p0 = hh * 64
                        ps = psSp.tile([128, 512], f32, name="psS", tag="psS")
                        nc.tensor.matmul(
                            ps[:], KT[j][p0:p0 + 64, ts(kt, 128)],
                            QT[j][p0:p0 + 64, qch])
                        u = utp.tile([128, 512], f16, name="ut", tag="ut")
                        nc.scalar.activation(u[:], ps[:], AF.Exp,
                                             bias=mk_t[kt])
                        out.append(u)
                    return out

                def emit_score_unit5(ci):
                    """Overflow scores: stationary [64, 128-replica-cols];
                    exp only the replica row-group this head's PV will use."""
                    j, qc = chunks[ci]
                    qch = ds(qc * 512, 512)
                    u5 = utp.tile([128, 512], f16, name="ut5", tag="ut5")
                    for hh in range(2):
                        p0 = hh * 64
                        base = ov * ((2 * j + hh) % nrep)
                        ps = psSp.tile([128, 512], f32, name="psS5", tag="psS")
                        nc.tensor.matmul(
                            ps[:], KT[j][p0:p0 + 64, ds(KF, 128)],
                            QT[j][p0:p0 + 64, qch])
                        nc.scalar.activation(
                            u5[base:base + ov, :], ps[base:base + ov, :],
                            AF.Exp, bias=mk5[base:base + ov, :])
                    return u5

                def emit_scores(ci):
                    ut = [emit_score_unit(ci, kt) for kt in range(kt_full)]
                    u5 = emit_score_unit5(ci) if ov else None
                    return (ut, u5)

                score_tiles = [None] * 6
                score_tiles[0] = emit_scores(0)

                # ---- VW = x_kept @ (Wv@Wo), sigma ones-column appended ----
                # psV nests inside psS and closes before psO opens: PSUM is
                # psS(8KB) + max(psV 4KB, psO 8KB) = 16KB exactly.
                with tc.tile_pool(name="psV", bufs=2, space="PSUM") as psV:
                    for m in range(nt):
                        xsrc = (xk5t if (ov and m == kt_full) else
                                [xkt[kt][:, ts(m, 128)] for kt in range(6)])
                        for ncn in range(2):
                            fch = ds(ncn * 384, 384)
                            ps = psV.tile([128, 384], f32, name="vw", tag="vw")
                            for kt in range(6):
                                src = xsrc[kt] if isinstance(xsrc, list) \
                                    else xsrc[kt]
                                nc.tensor.matmul(
                                    ps[:], src, wvo_t[kt][:, fch],
                                    start=(kt == 0),
                                    stop=(kt == 5 and not has_cvec))
                            if has_cvec:
                                nc.tensor.matmul(
                                    ps[:], xkt6[:], wvo6[:, fch],
                                    start=False, stop=True)
                            nc.vector.tensor_copy(VW[m][:, fch], ps[:])
                        nc.vector.memset(VW[m][:, 768:769], 1.0)

                # ---- attention main loop (pipelined) ----
                out_rings = [nc.sync, nc.gpsimd]
                ring_i = [0]

                def emit_pv_group(ci, mq, ut, u5):
                    """One 128-query block, both heads (A,B) interleaved so
                    the overflow MMs are adjacent (concurrent row groups)."""
                    j, qc = chunks[ci]
                    mqs = ts(mq, 128)
                    ps = {}
                    for half, fch in (("b", ds(384, 385)), ("a", ds(0, 384))):
                        w = 385 if half == "b" else 384
                        for hh in range(2):
                            p = psOp.tile([128, w], f32, name=f"ps{half}{hh}",
                                          tag=f"ps{half}")
                            ps[half, hh] = p
                            for kt in range(kt_full):
                                nc.tensor.matmul(
                                    p[:], ut[kt][hh][:, mqs], VW[kt][:, fch],
                                    start=(kt == 0),
                                    stop=(kt == kt_full - 1 and not ov))
                        if ov:
                            for hh in range(2):
                                base = ov * ((2 * j + hh) % nrep)
                                nc.tensor.matmul(
                                    ps[half, hh][:],
                                    u5[base:base + ov, mqs],
                                    VW[kt_full][base:base + ov, fch],
                                    start=False, stop=True,
                                    tile_position=(base, 0))
                        if half == "b":
                            for hh in range(2):
                                rv = ep.tile([128, 1], f32, name="rinv",
                                             tag="rinv")
                                nc.vector.reciprocal(
                                    rv[:], ps["b", hh][:, 384:385])
                                ps["rv", hh] = rv
                    for hh in range(2):
                        head = j * 2 + hh
                        ob = op_.tile([128, 768], f16, name="ob", tag="ob")
                        nc.vector.tensor_scalar_mul(
                            ob[:, 384:768], ps["b", hh][:, 0:384],
                            ps["rv", hh][:])
                        nc.vector.tensor_scalar_mul(
                            ob[:, 0:384], ps["a", hh][:], ps["rv", hh][:])
                        orow = out_d[head, ds(qc * 512 + mq * 128, 128), :]
                        out_rings[ring_i[0] % 2].dma_start(orow[:], ob[:])
                        ring_i[0] += 1

                # sprinkle schedule: during chunk c's 4 PV groups, emit chunk
                # c+1's score units (kt_full full pairs + 1 overflow pair).
                with tc.tile_pool(name="psO", bufs=2, space="PSUM") as psOp:
                    for ci in range(6):
                        ut, u5 = score_tiles[ci]
                        nunits = kt_full + (1 if ov else 0)
                        nxt = ([], None)
                        for mq in range(4):
                            emit_pv_group(ci, mq, ut, u5)
                            if ci + 1 < 6:
                                lo = (nunits * mq) // 4
                                hi = (nunits * (mq + 1)) // 4
                                for unit in range(lo, hi):
                                    if unit < kt_full:
                                        nxt[0].append(
                                            emit_score_unit(ci + 1, unit))
                                    else:
                                        nxt = (nxt[0],
                                               emit_score_unit5(ci + 1))
                        if ci + 1 < 6:
                            score_tiles[ci + 1] = nxt
    nc.compile()
    return nc


def get_program(kt_full, ov, has_cvec):
    key = (kt_full, ov, has_cvec)
    if key not in _PROGRAM_CACHE:
        _PROGRAM_CACHE[key] = _build_program(*key)
    return _PROGRAM_CACHE[key]


def prep(x, mask, Wq, bq, Wk, bk, Wv, bv, Wo, bo):
    """Host-side sharding/compaction.
    Tokens are permuted per batch so unmasked keys come first; the device
    computes in permuted token order and gather_output undoes it."""
    f16 = np.float16
    x = np.asarray(x, np.float32)
    mask = np.asarray(mask)
    Wq = np.asarray(Wq, np.float32)
    Wk = np.asarray(Wk, np.float32)
    Wv = np.asarray(Wv, np.float32)
    Wo = np.asarray(Wo, np.float32)
    bq = np.asarray(bq, np.float32)
    bk = np.asarray(bk, np.float32)
    bv = np.asarray(bv, np.float32)
    bo = np.asarray(bo, np.float32)

    mrow = [mask[b, 0, 0] != 0 for b in range(B)]
    perms = [np.argsort(~mrow[b], kind="stable") for b in range(B)]
    nkeep = [int(mrow[b].sum()) for b in range(B)]
    nkmax = max(1, max(nkeep))
    kt_full, r = divmod(nkmax, 128)
    if kt_full == 0:
        kt_full, r = 1, 0
    if r == 0:
        ov = 0
    elif r <= 32:
        ov = 32
    elif r <= 64:
        ov = 64
    else:
        kt_full, ov = kt_full + 1, 0
    KF = 128 * kt_full
    nrep = (128 // ov) if ov else 0
    nt = kt_full + (1 if ov else 0)

    cvec = bv @ Wo + bo
    has_cvec = bool(np.any(cvec))

    # per-head-group packed weights (shared across the 4 batches)
    wq_p, wk_p, bq_p, bk_p = [], [], [], []
    for g in range(2):
        cs = slice(g * GW, (g + 1) * GW)
        wq_p.append(_pack6((Wq[:, cs] * 0.125).astype(f16)))
        wk_p.append(_pack6(Wk[:, cs].astype(f16)))
        bq_p.append((bq[cs] * 0.125).reshape(3, 128).T)   # [128,3]
        bk_p.append(bk[cs].reshape(3, 128).T)
    wvp = _pack6((Wv @ Wo).astype(f16))
    wvo6 = cvec.astype(f16).reshape(1, 768)

    xp_b, xk5_b, sv_b = [], [], []
    for b in range(B):
        xpm = x[b][perms[b]].T.astype(f16)        # [768, 1024] permuted
        xp_b.append(_pack6(xpm))
        sv = np.empty((128, 6 + nt), np.float32)
        mkf = np.full(KF, -1e9, np.float32)
        mkf[:min(nkeep[b], KF)] = 0.0
        sv[:, 6:6 + kt_full] = mkf.reshape(kt_full, 128).T
        if ov:
            g = xpm[:, KF:KF + ov]                # [768, ov] overflow tokens
            xk5_b.append(_pack6(np.ascontiguousarray(np.tile(g, (1, nrep)))))
            nk5 = min(max(nkeep[b] - KF, 0), ov)
            m5 = np.full(ov, -1e9, np.float32)
            m5[:nk5] = 0.0
            sv[:, 6 + kt_full] = np.tile(m5, nrep)
        else:
            xk5_b.append(None)
        sv_b.append(sv)

    in_maps = []
    for c in range(NCORES):
        b, g = c // 2, c % 2
        sv = sv_b[b].copy()
        sv[:, 0:3] = bq_p[g]
        sv[:, 3:6] = bk_p[g]
        im = {
            "xp": xp_b[b],
            "wqp": wq_p[g],
            "wkp": wk_p[g],
            "wvp": wvp,
            "wvo6": wvo6,
            "sv": sv,
        }
        if ov:
            im["xk5"] = xk5_b[b]
        in_maps.append(im)
    return (kt_full, ov, has_cvec), in_maps, perms


def gather_output(results, perms):
    out = np.empty((B, S * NH, H), np.float32)
    ov = out.reshape(B, S, NH, H)
    for c in range(NCORES):
        b, g = c // 2, c % 2
        o = results[c]["out"]  # [6, 1024(permuted q), 768] fp16
        ov[b, perms[b], g * 6:(g + 1) * 6, :] = \
            o.transpose(1, 0, 2).astype(np.float32)
    return out


def kernel(**inputs):
    from concourse.bass_utils import run_bass_kernel_spmd

    cfg, in_maps, perms = prep(**inputs)
    nc = get_program(*cfg)
    res = run_bass_kernel_spmd(nc, in_maps, core_ids=list(range(NCORES)))
    return gather_output(res.results, perms)


if __name__ == "__main__":
    rng = np.random.default_rng(0)
    demo = {
        "x": rng.standard_normal((B, S, H), dtype=np.float32),
        "mask": rng.integers(0, 2, (B, 1, 1, S)).astype(np.int32),
        "Wq": rng.standard_normal((H, H), dtype=np.float32) / np.sqrt(H),
        "bq": np.zeros(H, np.float32),
        "Wk": rng.standard_normal((H, H), dtype=np.float32) / np.sqrt(H),
        "bk": np.zeros(H, np.float32),
        "Wv": rng.standard_normal((H, H), dtype=np.float32) / np.sqrt(H),
        "bv": np.zeros(H, np.float32),
        "Wo": rng.standard_normal((H, H), dtype=np.float32) / np.sqrt(H),
        "bo": np.zeros(H, np.float32),
    }
    out = kernel(**demo)
    print("kernel ran, output shape", out.shape)
